# revision 2
# baseline (speedup 1.0000x reference)
"""Trainium2 Bass kernel for a single-step GRU attention decoder.

Math (matches the reference nn.Module):
    xe  = emb[x]                                   # [H]
    a   = log_softmax(cat(xe, h0) @ attn_W.T + attn_b)   # [L]
    ap  = a @ encoder_outputs                      # [H]
    g   = relu(cat(xe, ap) @ ctx_W.T + ctx_b)      # [H]
    GRU(g, h0) -> h_new                            # [H]
    logits = h_new @ out_W.T + out_b               # [V]

Distribution across 8 NeuronCores (one TRN2 chip):
  - attention sharded over L (512 rows/core); exploiting linearity,
    log_softmax @ enc == a @ enc - (log sum exp a) * colsum(enc), so one
    AllReduce of per-core partials {a@enc, colsum(enc), sum(exp a)} lets
    every core reconstruct attn_applied locally.
  - ctx projection sharded by output rows; GRU mats sharded by *input*
    columns so a single AllReduce of partial (gi, gh) lets every core
    compute the full gates / h_new locally.
  - out projection sharded over vocab (6283 rows/core, padded to 6400).

Precision: the big weight planes are fp8 e3m4 (4 mantissa bits — ~2x
lower quantization error than e4m3 on these uniform(-s,s) weights, and
half the HBM bytes of bf16).  Weights are pre-scaled by 256 so they sit
in e3m4's normal range; residual "lo" planes are boosted by a further
64x to clear the subnormal floor, and the 1/64 / 1/256 fixups are folded
into rhs columns / activation scales / one tensor_scalar.  attn_W stays
bf16 (log-softmax amplifies attention-logit error through the huge
attn_applied activations and the saturated GRU gates).  PSUM accumulation
is fp32 throughout, as are softmax/gates/biases.

Schedule notes (from perfetto traces):
  - all small host tensors ride in two merged DMAs so the critical
    attn_wt/enc loads are issued ~4us earlier;
  - out_W is stored in PE consumption order (vocab-tile major) and its 5
    chunked loads are gated behind the attention exp on gpsimd, keeping
    the first ~17us of DMA bandwidth for the attention operands;
  - collective bounce copies ride the scalar-engine HWDGE ring so they
    never queue behind bulk loads on the sync ring;
  - the cross-partition sum of exp(a) uses a 1-column PE matmul instead
    of the (very slow) gpsimd C-axis reduce.
"""

import ml_dtypes
import numpy as np

import concourse.bass as bass
import concourse.bacc as bacc
import concourse.tile as tile
from concourse import mybir
from concourse.bass_utils import run_bass_kernel_spmd

H = 1024
V = 50257
L = 4096
NCORES = 8
LC = L // NCORES          # 512 encoder rows per core
HC = H // NCORES          # 128 hidden chunk per core
VC = -(-V // NCORES)      # 6283 vocab rows per core
VT = 50                   # vocab tiles of 128 per core
VPAD = VT * 128           # 6400
SC = 256.0                # e3m4 weight pre-scale
LS = 64.0                 # lo-plane subnormal boost
F32 = mybir.dt.float32
BF16 = mybir.dt.bfloat16
E3 = mybir.dt.float8e3
NPBF16 = ml_dtypes.bfloat16
NPE3 = ml_dtypes.float8_e3m4
RG = [list(range(NCORES))]

_CACHE = {}


def _build(dbg=False):
    key = ("nc", dbg)
    if key in _CACHE:
        return _CACHE[key]

    nc = bacc.Bacc("TRN2", target_bir_lowering=False, debug=False,
                   num_devices=NCORES)

    def din(name, shape, dt=F32):
        return nc.dram_tensor(name, shape, dt, kind="ExternalInput")

    smallb_d = din("smallb", [128, 17], BF16)  # 0:16 cat(xe,h0), 16 h0 chunk
    smallf_d = din("smallf", [128, 111])       # abias4 cbias1 h0cm8 bias8_48 obias50
    attn_wt_d = din("attn_wt", [128, 16 * LC], BF16)
    enc_hi_d = din("enc_hi", [128, 4 * H], E3)
    enc_lo_d = din("enc_lo", [128, 4 * H], E3)   # (enc - hi) * 64
    ctx_hi_d = din("ctx_hi", [128, 2048], E3)    # ctx_W.T * 256
    ctx_lo_d = din("ctx_lo", [128, 1024], E3)    # residual * 64 (ap half)
    wih_hi_d = din("wih_hi", [HC, 3 * H], E3)    # W_ih.T * 256
    wih_lo_d = din("wih_lo", [HC, 3 * H], E3)    # residual * 64
    whh_d = din("whh", [HC, 3 * H], E3)          # W_hh.T * 256
    out_wt_d = din("out_wt", [128, VT * 8 * 128], E3)  # [(t,j,q)] * 256
    out_d = nc.dram_tensor("out", [128, VT], F32, kind="ExternalOutput")

    AR = "AllReduce"
    ADD = mybir.AluOpType.add
    MUL = mybir.AluOpType.mult
    ACTF = mybir.ActivationFunctionType

    with tile.TileContext(nc) as tc:
        with (
            tc.tile_pool(name="wp", bufs=1) as wp,
            tc.tile_pool(name="sp", bufs=1) as sp,
            tc.tile_pool(name="pp", bufs=1, space="PSUM") as pp,
            tc.tile_pool(name="dp", bufs=1, space="DRAM") as dp,
        ):
            # ------------- loads (issue order = priority order) -------------
            smallb = sp.tile([128, 17], BF16, tag="smallb")
            nc.sync.dma_start(smallb[:], smallb_d[:])
            smallf = sp.tile([128, 111], F32, tag="smallf")
            nc.sync.dma_start(smallf[:], smallf_d[:])
            catin = smallb[:, 0:16]
            h0c = smallb[:, 16:17]
            abias = smallf[:, 0:4]
            cbias = smallf[:, 4:5]
            h0cm = smallf[:, 5:13]
            bias8 = smallf[:, 13:61]
            obias = smallf[:, 61:111]

            attn_sb = wp.tile([128, 16 * LC], BF16, tag="attn")
            nc.sync.dma_start(attn_sb[:, :8 * LC], attn_wt_d[:, :8 * LC])
            nc.sync.dma_start(attn_sb[:, 8 * LC:], attn_wt_d[:, 8 * LC:])
            enc_hi = wp.tile([128, 4 * H], E3, tag="enchi")
            nc.sync.dma_start(enc_hi[:], enc_hi_d[:])
            enc_lo = wp.tile([128, 4 * H], E3, tag="enclo")
            nc.sync.dma_start(enc_lo[:], enc_lo_d[:])
            ctx_hi = wp.tile([128, 2048], E3, tag="ctxhi")
            nc.sync.dma_start(ctx_hi[:], ctx_hi_d[:])
            ctx_lo = wp.tile([128, 1024], E3, tag="ctxlo")
            nc.sync.dma_start(ctx_lo[:], ctx_lo_d[:])
            wih_hi = wp.tile([128, 3 * H], E3, tag="wihhi")
            nc.sync.dma_start(wih_hi[:], wih_hi_d[:])
            wih_lo = wp.tile([128, 3 * H], E3, tag="wihlo")
            nc.sync.dma_start(wih_lo[:], wih_lo_d[:])
            whh_sb = wp.tile([128, 3 * H], E3, tag="whh")
            nc.sync.dma_start(whh_sb[:], whh_d[:])

            # out_W: PE-consumption-ordered, loaded in 5 chunks, gated
            # behind the attention exp (below) so the attention operands
            # own the DMA engines for the first ~17us.
            outw = wp.tile([128, VT * 8 * 128], E3, tag="outw")

            ones_c = sp.tile([128, 1], F32, tag="ones_c")
            nc.vector.memset(ones_c[:], 1.0)
            ones_r = sp.tile([1, 128], F32, tag="ones_r")
            nc.vector.memset(ones_r[:], 1.0)

            # ---------------- attention logits ----------------
            # a[l] for the 512 local l, laid out [128, 4] col-major tiles.
            # NOTE: accumulation groups must be contiguous in program order —
            # interleaving groups within one PSUM bank gives wrong results.
            # Split the contraction in two halves (xe tiles / h0 tiles) so the
            # PE can start as soon as the first half of attn_wt lands.
            a_psA = pp.tile([128, 4], F32, tag="apsA")
            for j in range(4):           # l tiles
                for i in range(8):       # xe half of the cat dim
                    nc.tensor.matmul(
                        a_psA[:, j:j + 1],
                        attn_sb[:, LC * i + 128 * j:LC * i + 128 * (j + 1)],
                        catin[:, i:i + 1],
                        start=(i == 0), stop=(i == 7))
            a_psB = pp.tile([128, 4], F32, tag="apsB")
            for j in range(4):           # l tiles
                for i in range(8, 16):   # h0 half
                    nc.tensor.matmul(
                        a_psB[:, j:j + 1],
                        attn_sb[:, LC * i + 128 * j:LC * i + 128 * (j + 1)],
                        catin[:, i:i + 1],
                        start=(i == 8), stop=(i == 15))
            a_half = sp.tile([128, 4], F32, tag="ahalf")
            nc.vector.tensor_add(a_half[:], a_psA[:], abias[:])
            a_sb = sp.tile([128, 4], F32, tag="a")
            nc.vector.tensor_add(a_sb[:], a_psB[:], a_half[:])
            exp_sb = sp.tile([128, 4], F32, tag="expa")
            nc.scalar.activation(exp_sb[:], a_sb[:], ACTF.Exp)

            # release the out_W loads now that the attention operands are
            # in SBUF (gpsimd executes in order: the gate copy waits on
            # exp_sb, then the issues run)
            gate = sp.tile([1, 1], F32, tag="gate")
            nc.gpsimd.tensor_copy(gate[:], exp_sb[0:1, 0:1])
            OCH = VT * 8 * 128 // 5      # 10240 cols = 10 vocab tiles
            for k in range(5):
                nc.gpsimd.dma_start(outw[:, OCH * k:OCH * (k + 1)],
                                    out_wt_d[:, OCH * k:OCH * (k + 1)])

            # split a into hi/lo bf16 rhs planes
            a_hi = sp.tile([128, 4], BF16, tag="ahi")
            nc.vector.tensor_copy(a_hi[:], a_sb[:])
            # rhs3 (for enc_hi): col 3j = a_hi_j, 3j+1 = a_lo_j, 3j+2 = 1
            rhs3 = sp.tile([128, 12], BF16, tag="rhs3")
            nc.vector.tensor_copy(rhs3[:, 0:12:3], a_hi[:])
            nc.vector.tensor_sub(rhs3[:, 1:12:3], a_sb[:], a_hi[:])
            nc.vector.memset(rhs3[:, 2:12:3], 1.0)
            # rhs2 (for enc_lo, which is 64x-boosted): a_hi/64, 1/64
            rhs2 = sp.tile([128, 8], BF16, tag="rhs2")
            nc.vector.tensor_scalar(rhs2[:, 0:8:2], a_hi[:], 1.0 / LS, None,
                                    MUL)
            nc.vector.memset(rhs2[:, 1:8:2], 1.0 / LS)

            # pack psum cols per h-chunk c: 5c+0 ehi*ahi, +1 ehi*alo,
            # +2 ehi*1, +3 elo*ahi, +4 elo*1   (elo cols pre-divided by 64
            # via rhs2, so all five are true-scale)
            pack_ps = pp.tile([128, 40], F32, tag="packps")
            for c in range(8):
                for j in range(4):
                    nc.tensor.matmul(
                        pack_ps[:, 5 * c:5 * c + 3],
                        enc_hi[:, H * j + 128 * c:H * j + 128 * (c + 1)],
                        rhs3[:, 3 * j:3 * j + 3],
                        start=(j == 0), stop=(j == 3))
                for j in range(4):
                    nc.tensor.matmul(
                        pack_ps[:, 5 * c + 3:5 * c + 5],
                        enc_lo[:, H * j + 128 * c:H * j + 128 * (c + 1)],
                        rhs2[:, 2 * j:2 * j + 2],
                        start=(j == 0), stop=(j == 3))
            # sum of exp(a): cols on vector, partitions via 1-col PE matmul
            exp_r = sp.tile([128, 1], F32, tag="expr")
            nc.vector.reduce_sum(exp_r[:], exp_sb[:], axis=mybir.AxisListType.X)
            ps_misc = pp.tile([128, 2], F32, tag="psmisc")
            nc.tensor.matmul(ps_misc[0:1, 0:1], exp_r[:], ones_c[:],
                             start=True, stop=True)

            pk_sb = sp.tile([128, 40], F32, tag="pksb")
            nc.vector.tensor_copy(pk_sb[:], pack_ps[:])
            pack_sb = sp.tile([128, 18], F32, tag="pack")
            nc.vector.tensor_add(pack_sb[:, 0:8], pk_sb[:, 0:40:5],
                                 pk_sb[:, 1:40:5])
            nc.vector.tensor_add(pack_sb[:, 0:8], pack_sb[:, 0:8],
                                 pk_sb[:, 3:40:5])
            nc.vector.tensor_add(pack_sb[:, 8:16], pk_sb[:, 2:40:5],
                                 pk_sb[:, 4:40:5])
            nc.vector.memset(pack_sb[:, 16:18], 0.0)
            nc.vector.tensor_copy(pack_sb[0:1, 16:17], ps_misc[0:1, 0:1])

            # gh = W_hh @ h0 depends only on inputs — run it here so the PE
            # does it inside the CC1 wait instead of on the critical path.
            gih_ps = pp.tile([128, 48], F32, tag="gihps")
            for c in range(24):
                nc.tensor.matmul(gih_ps[:, 24 + c:25 + c],
                                 whh_sb[:, 128 * c:128 * (c + 1)], h0c,
                                 start=True, stop=True)

            # The xe half of the ctx projection is also CC-independent, but
            # Tile gates an accumulation group on ALL its inputs — so give it
            # its own group (psum col 0) that completes pre-CC1, and stash it
            # in SBUF.
            g_ps = pp.tile([128, 2], F32, tag="gps")
            for i in range(8):
                nc.tensor.matmul(g_ps[:, 0:1], ctx_hi[:, 128 * i:128 * (i + 1)],
                                 catin[:, i:i + 1],
                                 start=(i == 0), stop=(i == 7))
            g_xe = sp.tile([128, 1], F32, tag="gxe")
            nc.vector.tensor_copy(g_xe[:], g_ps[:, 0:1])

            # ---------------- collective 1: AllReduce(add) ----------------
            cc1_in = dp.tile([128, 18], F32, tag="cc1in")
            cc1_out = dp.tile([128, 18], F32, tag="cc1out",
                              addr_space="Shared")
            nc.scalar.dma_start(cc1_in[:], pack_sb[:])
            nc.gpsimd.collective_compute(AR, ADD, replica_groups=RG,
                                         ins=[cc1_in.opt()],
                                         outs=[cc1_out.opt()])
            sums = sp.tile([128, 18], F32, tag="sums")
            nc.scalar.dma_start(sums[:], cc1_out[:])

            # c = log(sum exp), broadcast to all partitions via PE
            c_sb = sp.tile([1, 1], F32, tag="c")
            nc.scalar.activation(c_sb[:], sums[0:1, 16:17], ACTF.Ln)
            nc.tensor.matmul(ps_misc[:, 1:2], ones_r[:], c_sb[:],
                             start=True, stop=True)
            cb_sb = sp.tile([128, 1], F32, tag="cb")
            nc.vector.tensor_copy(cb_sb[:], ps_misc[:, 1:2])

            # attn_applied = partial1_sum - c * colsum_sum   [128, 8]
            atmp = sp.tile([128, 8], F32, tag="atmp")
            nc.vector.tensor_scalar(atmp[:], sums[:, 8:16], cb_sb[:], None,
                                    MUL)
            attnap = sp.tile([128, 8], F32, tag="attnap")
            nc.vector.tensor_sub(attnap[:], sums[:, 0:8], atmp[:])
            # split attnap hi/lo bf16 (lo fused via mixed-dtype sub)
            ap_hi = sp.tile([128, 8], BF16, tag="aphi")
            nc.vector.tensor_copy(ap_hi[:], attnap[:])

            # ---------------- context projection (attnap half) ----------
            # one accumulation group in psum col 1; the hi*ap_hi terms start
            # while the lo/64 and ap_lo columns are still being prepared.
            for m in range(8):
                nc.tensor.matmul(g_ps[:, 1:2],
                                 ctx_hi[:, 128 * (8 + m):128 * (9 + m)],
                                 ap_hi[:, m:m + 1],
                                 start=(m == 0), stop=False)
            ap_lo = sp.tile([128, 8], BF16, tag="aplo")
            nc.vector.tensor_sub(ap_lo[:], attnap[:], ap_hi[:])
            ap_h64 = sp.tile([128, 8], BF16, tag="aph64")
            nc.vector.tensor_scalar(ap_h64[:], ap_hi[:], 1.0 / LS, None, MUL)
            for m in range(8):
                nc.tensor.matmul(g_ps[:, 1:2],
                                 ctx_lo[:, 128 * m:128 * (m + 1)],
                                 ap_h64[:, m:m + 1],
                                 start=False, stop=False)
            for m in range(8):
                nc.tensor.matmul(g_ps[:, 1:2],
                                 ctx_hi[:, 128 * (8 + m):128 * (9 + m)],
                                 ap_lo[:, m:m + 1],
                                 start=False, stop=(m == 7))
            gpre = sp.tile([128, 1], F32, tag="gpre")
            nc.vector.tensor_add(gpre[:], g_ps[:, 1:2], g_xe[:])
            # g = relu(gpre/256 + ctx_b) — the 1/256 undoes the weight scale
            g_sb = sp.tile([128, 1], F32, tag="g")
            nc.scalar.activation(g_sb[:], gpre[:], ACTF.Relu, bias=cbias[:],
                                 scale=1.0 / SC)
            g_hi = sp.tile([128, 1], BF16, tag="ghi")
            nc.vector.tensor_copy(g_hi[:], g_sb[:])
            g_lo = sp.tile([128, 1], BF16, tag="glo")
            nc.vector.tensor_sub(g_lo[:], g_sb[:], g_hi[:])
            g_h64 = sp.tile([128, 1], BF16, tag="gh64")
            nc.vector.tensor_scalar(g_h64[:], g_hi[:], 1.0 / LS, None, MUL)

            # ---------------- GRU partial matvecs (gi; gh ran earlier) ----
            for c in range(24):
                whi = wih_hi[:, 128 * c:128 * (c + 1)]
                nc.tensor.matmul(gih_ps[:, c:c + 1], whi, g_hi[:],
                                 start=True, stop=False)
                nc.tensor.matmul(gih_ps[:, c:c + 1], whi, g_lo[:],
                                 start=False, stop=False)
                nc.tensor.matmul(gih_ps[:, c:c + 1],
                                 wih_lo[:, 128 * c:128 * (c + 1)], g_h64[:],
                                 start=False, stop=True)
            # undo the 256x weight scale, fold in biases/8 (the 8-way
            # AllReduce sum restores them)
            t48 = sp.tile([128, 48], F32, tag="t48")
            nc.vector.tensor_scalar(t48[:], gih_ps[:], 1.0 / SC, None, MUL)
            pack2 = sp.tile([128, 48], F32, tag="pack2")
            nc.vector.tensor_add(pack2[:], t48[:], bias8[:])

            # ---------------- collective 2: AllReduce(add) ----------------
            cc2_in = dp.tile([128, 48], F32, tag="cc2in")
            cc2_out = dp.tile([128, 48], F32, tag="cc2out",
                              addr_space="Shared")
            nc.scalar.dma_start(cc2_in[:], pack2[:])
            nc.gpsimd.collective_compute(AR, ADD, replica_groups=RG,
                                         ins=[cc2_in.opt()],
                                         outs=[cc2_out.opt()])
            gsum = sp.tile([128, 48], F32, tag="gsum")
            nc.scalar.dma_start(gsum[:], cc2_out[:])

            # gates (PyTorch order r, z, n); r and z share one Sigmoid pass
            rzpre = sp.tile([128, 16], F32, tag="rzpre")
            nc.vector.tensor_add(rzpre[:], gsum[:, 0:16], gsum[:, 24:40])
            rz_sb = sp.tile([128, 16], F32, tag="rz")
            nc.scalar.activation(rz_sb[:], rzpre[:], ACTF.Sigmoid)
            npre = sp.tile([128, 8], F32, tag="npre")
            nc.vector.tensor_mul(npre[:], rz_sb[:, 0:8], gsum[:, 40:48])
            nc.vector.tensor_add(npre[:], npre[:], gsum[:, 16:24])
            n_sb = sp.tile([128, 8], F32, tag="n")
            nc.scalar.activation(n_sb[:], npre[:], ACTF.Tanh)
            # h_new = n + z * (h0 - n); pre-divide by 256 so the e3m4
            # out_W (x256) cancels and the psum lands at true scale
            hd = sp.tile([128, 8], F32, tag="hd")
            nc.vector.tensor_sub(hd[:], h0cm[:], n_sb[:])
            nc.vector.tensor_mul(hd[:], hd[:], rz_sb[:, 8:16])
            hnf = sp.tile([128, 8], F32, tag="hnf")
            nc.vector.tensor_add(hnf[:], n_sb[:], hd[:])
            hnew_b = sp.tile([128, 8], BF16, tag="hnewb")
            nc.vector.tensor_scalar(hnew_b[:], hnf[:], 1.0 / SC, None, MUL)

            # ---------------- output projection ----------------
            logit_sb = sp.tile([128, VT], F32, tag="logit")
            t0c = 0
            while t0c < VT:
                nt = min(8, VT - t0c)
                o_ps = pp.tile([128, nt], F32, tag="ops", bufs=2)
                for t in range(t0c, t0c + nt):
                    for j in range(8):
                        nc.tensor.matmul(
                            o_ps[:, t - t0c:t - t0c + 1],
                            outw[:, (t * 8 + j) * 128:(t * 8 + j + 1) * 128],
                            hnew_b[:, j:j + 1],
                            start=(j == 0), stop=(j == 7))
                nc.vector.tensor_add(logit_sb[:, t0c:t0c + nt], o_ps[:],
                                     obias[:, t0c:t0c + nt])
                t0c += nt
                if t0c == 24:
                    nc.scalar.dma_start(out_d[:, 0:24], logit_sb[:, 0:24])
            nc.scalar.dma_start(out_d[:, 24:VT], logit_sb[:, 24:VT])

            if dbg:
                dbg_tiles = {
                    "dbg_a": a_sb, "dbg_exp": exp_sb, "dbg_pack": pack_sb,
                    "dbg_sums": sums, "dbg_cb": cb_sb, "dbg_attnap": attnap,
                    "dbg_g": g_sb, "dbg_pack2": pack2, "dbg_gsum": gsum,
                    "dbg_hnew": hnew_b,
                }
                for name, t in dbg_tiles.items():
                    shp = list(t[:].shape)
                    d = nc.dram_tensor(name, shp, t[:].dtype,
                                       kind="ExternalOutput")
                    nc.sync.dma_start(d[:], t[:])

    nc.compile()
    _CACHE[key] = nc
    return nc


def _col_major(v, ncols):
    # v [n] -> [128, ncols] with [p, c] = v[128 * c + p]
    return np.ascontiguousarray(v.reshape(ncols, 128).T)


def _pack_rows(a, nb):
    # a [nb*128, w] -> [128, nb*w] with [p, w*i + q] = a[128*i + p, q]
    w = a.shape[1]
    return np.ascontiguousarray(
        a.reshape(nb, 128, w).transpose(1, 0, 2).reshape(128, nb * w))


def _e3_hi_lo(a):
    hi = a.astype(NPE3)
    lo = ((a - hi.astype(np.float32)) * LS).astype(NPE3)
    return hi, lo


def _shard(inputs):
    x = np.asarray(inputs["x"]).reshape(-1)
    h0 = np.asarray(inputs["h"], dtype=np.float32).reshape(H)
    enc = np.asarray(inputs["encoder_outputs"], dtype=np.float32)
    emb = np.asarray(inputs["emb"])
    attn_W = np.asarray(inputs["attn_W"], dtype=np.float32)
    attn_b = np.asarray(inputs["attn_b"], dtype=np.float32)
    ctx_W = np.asarray(inputs["ctx_W"], dtype=np.float32)
    ctx_b = np.asarray(inputs["ctx_b"], dtype=np.float32)
    W_ih = np.asarray(inputs["W_ih"], dtype=np.float32)
    W_hh = np.asarray(inputs["W_hh"], dtype=np.float32)
    b_ih = np.asarray(inputs["b_ih"], dtype=np.float32)
    b_hh = np.asarray(inputs["b_hh"], dtype=np.float32)
    out_W = np.asarray(inputs["out_W"], dtype=np.float32)
    out_b = np.asarray(inputs["out_b"], dtype=np.float32)

    xe = np.asarray(emb[int(x[0])], dtype=np.float32)
    smallb = np.concatenate(
        [_col_major(xe, 8), _col_major(h0, 8), np.zeros((128, 1))],
        axis=1).astype(NPBF16)
    h0cm = _col_major(h0, 8)
    bias8 = np.concatenate([_col_major(b_ih, 24), _col_major(b_hh, 24)],
                           axis=1) / 8.0

    in_maps = []
    for k in range(NCORES):
        lsl = slice(LC * k, LC * (k + 1))
        hsl = slice(HC * k, HC * (k + 1))
        v0, v1 = VC * k, min(VC * (k + 1), V)
        # out_W in PE consumption order: [p, (t*8 + j)*128 + q] =
        # 256 * out_W[v0 + 128t + q, 128j + p]
        owt = np.zeros((H, VPAD), dtype=np.float32)
        owt[:, :v1 - v0] = out_W[v0:v1, :].T * SC
        owt = np.ascontiguousarray(
            owt.reshape(8, 128, VT, 128).transpose(1, 2, 0, 3)
               .reshape(128, VT * 8 * 128)).astype(NPE3)
        ob = np.zeros(VPAD, dtype=np.float32)
        ob[:v1 - v0] = out_b[v0:v1]

        enc_hi, enc_lo = _e3_hi_lo(enc[lsl, :])
        ctxT = np.ascontiguousarray(ctx_W[hsl, :].T) * SC   # [2048, 128]
        ctx_hi = ctxT.astype(NPE3)
        ctx_lo = ((ctxT[H:] - ctx_hi[H:].astype(np.float32)) * LS
                  ).astype(NPE3)
        wihT = np.ascontiguousarray(W_ih[:, hsl].T) * SC    # [128, 3072]
        wih_hi, wih_lo = _e3_hi_lo(wihT)

        sb = smallb.copy()
        sb[:, 16] = h0[hsl].astype(NPBF16)
        smallf = np.concatenate([
            _col_major(attn_b[lsl], 4), ctx_b[hsl].reshape(128, 1),
            h0cm, bias8, _col_major(ob, VT)], axis=1)

        in_maps.append({
            "smallb": np.ascontiguousarray(sb),
            "smallf": np.ascontiguousarray(smallf, dtype=np.float32),
            "attn_wt": _pack_rows(attn_W[lsl, :].T.astype(NPBF16), 16),
            "enc_hi": _pack_rows(enc_hi, 4),
            "enc_lo": _pack_rows(enc_lo, 4),
            "ctx_hi": _pack_rows(ctx_hi, 16),
            "ctx_lo": _pack_rows(ctx_lo, 8),
            "wih_hi": wih_hi,
            "wih_lo": wih_lo,
            "whh": (np.ascontiguousarray(W_hh[:, hsl].T) * SC).astype(NPE3),
            "out_wt": owt,
        })
    return in_maps


def _gather(results):
    logits = np.empty(NCORES * VC, dtype=np.float32)
    for k in range(NCORES):
        chunk = np.asarray(results[k]["out"]).T.ravel()   # [VT*128]
        logits[VC * k:VC * (k + 1)] = chunk[:VC]
    return logits[:V].reshape(1, V)


def kernel(**inputs):
    nc = _build()
    in_maps = _shard(inputs)
    try:
        res = run_bass_kernel_spmd(nc, in_maps, core_ids=list(range(NCORES)))
    except Exception:
        # A dirty device state from a previous process occasionally fails
        # the first launch (NRT_EXEC_UNIT_UNRECOVERABLE); one retry clears.
        res = run_bass_kernel_spmd(nc, in_maps, core_ids=list(range(NCORES)))
    return _gather(res.results)


def kernel_traced(**inputs):
    """Like kernel() but profiles on HW; returns (output, exec_time_ns)."""
    nc = _build()
    in_maps = _shard(inputs)
    res = run_bass_kernel_spmd(nc, in_maps, core_ids=list(range(NCORES)),
                               trace=True)
    return _gather(res.results), res.exec_time_ns


def kernel_debug(**inputs):
    """Run the debug build; returns per-core dicts of all outputs."""
    nc = _build(dbg=True)
    in_maps = _shard(inputs)
    res = run_bass_kernel_spmd(nc, in_maps, core_ids=list(range(NCORES)))
    return res.results
